# revision 6
# baseline (speedup 1.0000x reference)
"""Trainium2 Bass kernel for DotProductAttentionLayer.

Computes, for inputs x [T, B, H], rand_ctx [T, B, H], W [H, H]:
  results = concat([x, where(t < 2, x, rand_ctx)], axis=-1)          [T, B, 2H]
  attn[b, t, s] = softmax_{s < t}((x_t W) . x_s), zero outside mask  [B, T, T]

Sharding: batch dim B=16 split across 8 NeuronCores (2 batches/core), W
replicated.  Per core and batch:
  XT = x^T (PE transposes)             [H, T]
  Y  = W^T x^T = Q^T (MM1)             [H, T]
  S  = Y^T-tiles @ XT  (MM2, causal)   [T, T] lower-triangular chunks
  attn = masked softmax rows of S
Matmuls run as 3-term bf16 splits (a_hi*b_hi + a_hi*b_lo + a_lo*b_hi) for
near-fp32 accuracy at bf16 PE throughput, or optionally single-pass fp32r.
"""

import os
import sys

sys.path.insert(0, "/opt/trn_rl_repo")

import numpy as np

T, B, H = 2048, 16, 512
NCORES = 8
BPC = B // NCORES          # batches per core
KT = H // 128              # 4 contraction tiles
TT = T // 128              # 16 row tiles
SC = T // 512              # 4 column (s) chunks
NEG = -1.0e30

MODE = os.environ.get("BASS_ATTN_MODE", "split3")  # "split3" | "f32r"


def build(t=T, bpc=BPC, mode=MODE, debug=False):
    from concourse import bacc, tile, mybir
    from concourse.masks import make_identity

    tt = t // 128
    sc = max(1, t // 512)
    cw = min(t, 512)          # chunk width
    tc_n = t // cw            # t-chunks for XT/Y production

    f32 = mybir.dt.float32
    bf16 = mybir.dt.bfloat16
    f32r = mybir.dt.float32r

    nc = bacc.Bacc(None, target_bir_lowering=False, debug=debug)
    x_d = nc.dram_tensor("x", [t, bpc, H], f32, kind="ExternalInput")
    rc_d = nc.dram_tensor("rc", [t, bpc, H], f32, kind="ExternalInput")
    w_d = nc.dram_tensor("w", [H, H], f32, kind="ExternalInput")
    if mode == "split3":
        wh_d = nc.dram_tensor("wh", [H, H], bf16, kind="ExternalInput")
        wl_d = nc.dram_tensor("wl", [H, H], bf16, kind="ExternalInput")
    msk_d = nc.dram_tensor("msk", [4, 128, cw], f32, kind="ExternalInput")
    res_d = nc.dram_tensor("res", [t, bpc, 2 * H], f32, kind="ExternalOutput")
    attn_d = nc.dram_tensor("attn", [bpc, t, t], f32, kind="ExternalOutput")

    from contextlib import ExitStack

    with tile.TileContext(nc) as tc, ExitStack() as es:
        consts = es.enter_context(tc.tile_pool(name="consts", bufs=1))
        xp = es.enter_context(tc.tile_pool(name="xp", bufs=6))
        rcp = es.enter_context(tc.tile_pool(name="rcp", bufs=4))
        xtp = es.enter_context(tc.tile_pool(name="xtp", bufs=1))
        yp_sb = es.enter_context(tc.tile_pool(name="yp_sb", bufs=1))
        stg = es.enter_context(tc.tile_pool(name="stg", bufs=3))
        stats = es.enter_context(tc.tile_pool(name="stats", bufs=8))
        trp = es.enter_context(tc.tile_pool(name="trp", bufs=2, space="PSUM"))
        yp_ps = es.enter_context(tc.tile_pool(name="yp_ps", bufs=2, space="PSUM"))
        sp = es.enter_context(tc.tile_pool(name="sp", bufs=4, space="PSUM"))

        ident = consts.tile([128, 128], f32)
        make_identity(nc, ident[:])
        zero_sb = consts.tile([128, cw], f32)
        nc.gpsimd.memset(zero_sb[:], 0.0)
        mask_sb = consts.tile([128, 4, cw], f32)
        nc.sync.dma_start(mask_sb[:], msk_d.rearrange("m p c -> p m c"))

        if mode == "split3":
            wh_sb = consts.tile([128, KT, H], bf16)
            wl_sb = consts.tile([128, KT, H], bf16)
            nc.sync.dma_start(wh_sb[:], wh_d.rearrange("(kt p) k -> p kt k", p=128))
            nc.sync.dma_start(wl_sb[:], wl_d.rearrange("(kt p) k -> p kt k", p=128))
        else:
            w_sb = consts.tile([128, KT, H], f32)
            nc.sync.dma_start(w_sb[:], w_d.rearrange("(kt p) k -> p kt k", p=128))
            wr_sb = consts.tile([128, KT, H], f32r)
            nc.gpsimd.tensor_copy(wr_sb[:], w_sb[:])

        for b in range(bpc):
            # ---------------- stage 1: load x, write results, build XT ----
            if mode == "split3":
                xth = xtp.tile([128, KT, t], bf16, tag="xth")
                xtl = xtp.tile([128, KT, t], bf16, tag="xtl")
            else:
                xtr = xtp.tile([128, KT, t], f32r, tag="xtr")
            for tcc in range(tc_n):
                xf = []
                for q in range(cw // 128):
                    ti = tcc * (cw // 128) + q
                    r0 = ti * 128
                    xt_ = xp.tile([128, H], f32, tag="x")
                    nc.sync.dma_start(xt_[:], x_d[r0:r0 + 128, b, :])
                    xf.append(xt_)
                    nc.sync.dma_start(res_d[r0:r0 + 128, b, 0:H], xt_[:])
                    rct = rcp.tile([128, H], f32, tag="rc")
                    nc.sync.dma_start(rct[:], rc_d[r0:r0 + 128, b, :])
                    if ti == 0:
                        nc.vector.tensor_copy(rct[0:2, :], xt_[0:2, :])
                    nc.sync.dma_start(res_d[r0:r0 + 128, b, H:2 * H], rct[:])
                for kt in range(KT):
                    ps = trp.tile([128, cw], f32, tag="tr")
                    for q in range(cw // 128):
                        nc.tensor.transpose(
                            ps[:, q * 128:(q + 1) * 128],
                            xf[q][:, kt * 128:(kt + 1) * 128],
                            ident[:],
                        )
                    csl = slice(tcc * cw, tcc * cw + cw)
                    if mode == "split3":
                        nc.scalar.copy(xth[:, kt, csl], ps[:])
                        nc.vector.tensor_sub(xtl[:, kt, csl], ps[:], xth[:, kt, csl])
                    else:
                        nc.scalar.copy(xtr[:, kt, csl], ps[:])

            # ---------------- stage 2: MM1  Y = W^T-contraction x^T -------
            if mode == "split3":
                yh = yp_sb.tile([128, KT, t], bf16, tag="yh")
                yl = yp_sb.tile([128, KT, t], bf16, tag="yl")
            else:
                yr = yp_sb.tile([128, KT, t], f32r, tag="yr")
            for mo in range(KT):
                msl = slice(mo * 128, (mo + 1) * 128)
                for tcp in range(0, tc_n, 2):
                    nps = min(2, tc_n - tcp)
                    pss = [yp_ps.tile([128, cw], f32, tag="y", name=f"yps{mo}_{tcp}_{t2}") for t2 in range(nps)]
                    for kt in range(KT):
                        if mode == "split3":
                            for t2 in range(nps):
                                csl = slice((tcp + t2) * cw, (tcp + t2 + 1) * cw)
                                nc.tensor.matmul(
                                    pss[t2][:], wh_sb[:, kt, msl], xth[:, kt, csl],
                                    start=(kt == 0), stop=False)
                                nc.tensor.matmul(
                                    pss[t2][:], wh_sb[:, kt, msl], xtl[:, kt, csl],
                                    start=False, stop=False)
                            for t2 in range(nps):
                                csl = slice((tcp + t2) * cw, (tcp + t2 + 1) * cw)
                                nc.tensor.matmul(
                                    pss[t2][:], wl_sb[:, kt, msl], xth[:, kt, csl],
                                    start=False, stop=(kt == KT - 1))
                        else:
                            for t2 in range(nps):
                                csl = slice((tcp + t2) * cw, (tcp + t2 + 1) * cw)
                                nc.tensor.matmul(
                                    pss[t2][:], wr_sb[:, kt, msl], xtr[:, kt, csl],
                                    start=(kt == 0), stop=(kt == KT - 1))
                    for t2 in range(nps):
                        csl = slice((tcp + t2) * cw, (tcp + t2 + 1) * cw)
                        if mode == "split3":
                            nc.scalar.copy(yh[:, mo, csl], pss[t2][:])
                            nc.vector.tensor_sub(yl[:, mo, csl], pss[t2][:], yh[:, mo, csl])
                        else:
                            nc.scalar.copy(yr[:, mo, csl], pss[t2][:])

            # ---------------- stage 3: MM2 + masked softmax ---------------
            for i in range(tt):
                nj = i // (cw // 128) + 1       # chunks needed (causal)
                jd = nj - 1                      # diagonal chunk index
                r = i % (cw // 128)
                isl = slice(i * 128, (i + 1) * 128)
                ssb = stg.tile([128, sc * cw], f32, tag="attn")
                pss = [sp.tile([128, cw], f32, tag="s", name=f"sps{i}_{j}") for j in range(nj)]
                for ko in range(KT):
                    if mode == "split3":
                        for j in range(nj):
                            jsl = slice(j * cw, (j + 1) * cw)
                            nc.tensor.matmul(
                                pss[j][:], yh[:, ko, isl], xth[:, ko, jsl],
                                start=(ko == 0), stop=False)
                            nc.tensor.matmul(
                                pss[j][:], yh[:, ko, isl], xtl[:, ko, jsl],
                                start=False, stop=False)
                        for j in range(nj):
                            jsl = slice(j * cw, (j + 1) * cw)
                            nc.tensor.matmul(
                                pss[j][:], yl[:, ko, isl], xth[:, ko, jsl],
                                start=False, stop=(ko == KT - 1))
                    else:
                        for j in range(nj):
                            jsl = slice(j * cw, (j + 1) * cw)
                            nc.tensor.matmul(
                                pss[j][:], yr[:, ko, isl], xtr[:, ko, jsl],
                                start=(ko == 0), stop=(ko == KT - 1))
                for j in range(nj):
                    jsl = slice(j * cw, (j + 1) * cw)
                    if j == jd:
                        nc.vector.tensor_add(ssb[:, jsl], pss[j][:], mask_sb[:, r, :])
                    else:
                        nc.scalar.copy(ssb[:, jsl], pss[j][:])
                nmax = stats.tile([128, 1], f32, tag="nmax")
                nc.vector.tensor_reduce(
                    nmax[:], ssb[:, 0:nj * cw], axis=mybir.AxisListType.X,
                    op=mybir.AluOpType.max, negate=True)
                sums = stats.tile([128, sc], f32, tag="sums")
                for j in range(nj):
                    jsl = slice(j * cw, (j + 1) * cw)
                    nc.scalar.activation(
                        out=ssb[:, jsl], in_=ssb[:, jsl],
                        func=mybir.ActivationFunctionType.Exp,
                        bias=nmax[:, 0:1], scale=1.0,
                        accum_out=sums[:, j:j + 1])
                tot = stats.tile([128, 1], f32, tag="tot")
                if nj > 1:
                    nc.vector.tensor_reduce(
                        tot[:], sums[:, 0:nj], axis=mybir.AxisListType.X,
                        op=mybir.AluOpType.add)
                else:
                    nc.vector.tensor_copy(tot[:], sums[:, 0:1])
                rec = stats.tile([128, 1], f32, tag="rec")
                nc.vector.reciprocal(rec[:], tot[:])
                nc.gpsimd.tensor_scalar_mul(ssb[:, 0:nj * cw], ssb[:, 0:nj * cw], rec[:, 0:1])
                if i == 0:
                    nc.gpsimd.memset(ssb[0:2, 0:nj * cw], 0.0)
                nc.sync.dma_start(attn_d[b, isl, 0:nj * cw], ssb[:, 0:nj * cw])
                for j in range(nj, sc):
                    jsl = slice(j * cw, (j + 1) * cw)
                    nc.sync.dma_start(attn_d[b, isl, jsl], zero_sb[:])

    nc.compile()
    return nc


def make_masks(cw=512):
    p = np.arange(128)[:, None]
    c = np.arange(cw)[None, :]
    m = np.zeros((4, 128, cw), dtype=np.float32)
    for r in range(4):
        m[r] = np.where(c < 128 * r + p, 0.0, NEG)
    return m


_built = {}


def _get_nc(mode=MODE):
    if mode not in _built:
        _built[mode] = build(mode=mode)
    return _built[mode]


def kernel(inputs, rand_ctx, W, attention_width=3):
    from concourse import bass_utils

    inputs = np.ascontiguousarray(inputs, dtype=np.float32)
    rand_ctx = np.ascontiguousarray(rand_ctx, dtype=np.float32)
    W = np.ascontiguousarray(W, dtype=np.float32)
    nc = _get_nc()
    msk = make_masks()
    in_maps = []
    for core in range(NCORES):
        bs = slice(core * BPC, (core + 1) * BPC)
        im = {
            "x": np.ascontiguousarray(inputs[:, bs, :]),
            "rc": np.ascontiguousarray(rand_ctx[:, bs, :]),
            "w": W,
            "msk": msk,
        }
        if MODE == "split3":
            import ml_dtypes
            wh = W.astype(ml_dtypes.bfloat16)
            wl = (W - wh.astype(np.float32)).astype(ml_dtypes.bfloat16)
            im["wh"] = wh
            im["wl"] = wl
        in_maps.append(im)
    res = bass_utils.run_bass_kernel_spmd(nc, in_maps, core_ids=list(range(NCORES)))
    results = np.concatenate([r["res"] for r in res.results], axis=1)
    attn = np.concatenate([r["attn"] for r in res.results], axis=0)
    return results, attn


if __name__ == "__main__":
    rng = np.random.default_rng(0)
    x = rng.standard_normal((T, B, H), dtype=np.float32)
    rc = rng.random((T, B, H), dtype=np.float32)
    W = rng.standard_normal((H, H), dtype=np.float32) * 0.06
    r, a = kernel(x, rc, W, 3)
    print(r.shape, a.shape)
